# revision 23
# baseline (speedup 1.0000x reference)
"""Cross-attention kernel for Trainium2, 8-core SPMD.

Problem (hardcoded shapes): B=4, N=4096, S=512, DIM=1024, H=16, D=64.
Sharding: data-parallel over B (4) x tensor-parallel over head-groups (2).
Each core computes 8 heads for one batch; host sums the two head-group
partial projection outputs per batch (partials stored fp16, summed fp32).

v2 structure (per 512-token chunk, 4 head-pair waves):
  wave w (heads 2w, 2w+1):
    scores: K=64-contraction matmuls for the two heads interleaved so the
      PE row-tiling (tile_position (0,0) / (64,0), tile_size 64x128) runs
      the pair CONCURRENTLY in the two array halves -> ~2x on scores.
    exp (ACT, 4x [128,1024]) -> e fp16
    AV: full-K [128,65] lhsT per head (65th col = m01 mask -> masked
      softmax denominator in psum row 64)
    normalize: bounce den rows, reciprocal_approx_fast, gpsimd
      partition_broadcast, DVE mult out of PSUM -> ot fp16
    + out-proj slice of chunk c-1 (ns=w) and q-proj slice of chunk c+1
      (jq=w) emitted per wave so the PE always has independent full-mode
      matmul work to fill scores/exp dependency stalls.

All matmul operands fp16 (1 col/cycle on the PE); fp32 accumulation.
"""
import numpy as np

P = 128
B, N, S, DIM = 4, 4096, 512, 1024
HEADS, D = 16, 64
HG = 8               # heads per core
GF = HG * D          # 512 features per head-group
NCHUNK = 512
NCH = N // NCHUNK    # 8 chunks
KT_TILES = DIM // P  # 8 contraction tiles for projections
JQ = GF // P         # 4 q-feature tiles
ST = S // P          # 4 s tiles
SCALE = D ** -0.5

LAST_RESULTS = None
_CACHED_NC = None


def _build():
    import concourse.mybir as mybir
    import concourse.tile as tile
    from concourse import bacc

    f32 = mybir.dt.float32
    f16 = mybir.dt.float16
    EXP = mybir.ActivationFunctionType.Exp
    IDENT = mybir.ActivationFunctionType.Identity

    nc = bacc.Bacc("TRN2", target_bir_lowering=False, debug=False)

    xT = nc.dram_tensor("xT", [DIM, N], f16, kind="ExternalInput")
    ctxT = nc.dram_tensor("ctxT", [DIM, S], f16, kind="ExternalInput")
    qw = nc.dram_tensor("qw", [DIM, GF], f16, kind="ExternalInput")
    kw = nc.dram_tensor("kw", [DIM, GF], f16, kind="ExternalInput")
    vw = nc.dram_tensor("vw", [DIM, GF], f16, kind="ExternalInput")
    pw = nc.dram_tensor("pw", [GF, DIM], f16, kind="ExternalInput")
    qb = nc.dram_tensor("qb", [P, JQ], f32, kind="ExternalInput")
    kb = nc.dram_tensor("kb", [P, JQ], f32, kind="ExternalInput")
    vb = nc.dram_tensor("vb", [1, GF], f32, kind="ExternalInput")
    m01 = nc.dram_tensor("m01", [P, ST], f32, kind="ExternalInput")
    o = nc.dram_tensor("o", [N, DIM], f16, kind="ExternalOutput")

    with tile.TileContext(nc) as tc:
        with (
            tc.tile_pool(name="const", bufs=1) as cpool,
            tc.tile_pool(name="kv", bufs=1) as kvpool,
            tc.tile_pool(name="qt", bufs=2) as qtpool,
            tc.tile_pool(name="xq", bufs=3) as xqpool,
            tc.tile_pool(name="e", bufs=2) as epool,
            tc.tile_pool(name="ot", bufs=3) as otpool,
            tc.tile_pool(name="nm", bufs=2) as nmpool,
            tc.tile_pool(name="ost", bufs=2) as ostpool,
            tc.tile_pool(name="ps_sc", bufs=1, space="PSUM") as ps_sc,
            tc.tile_pool(name="ps_av", bufs=2, space="PSUM") as ps_av,
            tc.tile_pool(name="ps_p", bufs=2, space="PSUM") as ps_p,
        ):
            # ---- loads in critical-path order: ctx+kw gate KT; qw+x0 gate
            # qproj(0); vw gates V; x1 gates qproj(1); pw needed only at the
            # first out-proj (~chunk 1) ----
            ctx_sb = xqpool.tile([P, KT_TILES, S], f16, tag="xq")
            kw_sb = cpool.tile([P, KT_TILES, GF], f16)
            kb_sb = cpool.tile([P, JQ], f32)
            qw_sb = cpool.tile([P, KT_TILES, GF], f16)
            qb_sb = cpool.tile([P, JQ], f32)
            vw_sb = cpool.tile([P, KT_TILES, GF], f16)
            vbm_sb = cpool.tile([P, ST, GF], f32)
            vb_sb = cpool.tile([1, GF], f32)
            vbb_sb = cpool.tile([P, GF], f32)
            m01_sb = cpool.tile([P, ST], f32)
            pw_sb = cpool.tile([P, GF // P, DIM], f16)

            # warm the PE HAM clock-gate with dummy matmuls while the input
            # DMAs land (the first ~3.4us of PE activity runs at 1.2GHz
            # otherwise)
            wsc = cpool.tile([P, P], f16)
            rsc = cpool.tile([P, NCHUNK], f16)
            nc.vector.memset(wsc[:], 0.0)
            nc.vector.memset(rsc[:], 0.0)
            wps = ps_p.tile([P, NCHUNK], f32, tag="proj")
            for _ in range(16):
                nc.tensor.matmul(wps[:], wsc[:], rsc[:], start=True, stop=True)

            kw_r = kw.rearrange("(ko ki) m -> ki ko m", ki=P)
            qw_r = qw.rearrange("(ko ki) m -> ki ko m", ki=P)
            ctxT_r = ctxT.rearrange("(ko ki) s -> ki ko s", ki=P)
            nc.sync.dma_start(ctx_sb[:], ctxT_r)
            nc.sync.dma_start(kw_sb[:, :, 0:P], kw_r[:, :, 0:P])
            nc.sync.dma_start(kb_sb[:], kb[:])
            for jk in range(1, JQ):
                nc.sync.dma_start(kw_sb[:, :, jk * P:(jk + 1) * P],
                                  kw_r[:, :, jk * P:(jk + 1) * P])

            xT_r = xT.rearrange("(ko ki) n -> ki ko n", ki=P)

            def emit_xq(c):
                xq = xqpool.tile([P, KT_TILES, NCHUNK], f16, tag="xq")
                lo = c * NCHUNK
                nc.sync.dma_start(xq[:], xT_r[:, :, lo:lo + NCHUNK])
                return xq

            xq_cur = emit_xq(0)
            nc.sync.dma_start(qw_sb[:, :, 0:P], qw_r[:, :, 0:P])
            nc.sync.dma_start(qb_sb[:], qb[:])
            for jq in range(1, JQ):
                nc.sync.dma_start(qw_sb[:, :, jq * P:(jq + 1) * P],
                                  qw_r[:, :, jq * P:(jq + 1) * P])

            nc.sync.dma_start(vw_sb[:], vw.rearrange("(ko ki) m -> ki ko m", ki=P))
            nc.sync.dma_start(vb_sb[:], vb[:])
            nc.sync.dma_start(m01_sb[:], m01[:])

            # ---- KT = kw.T @ ctxT -> [128, JQ, S] (k-features on partitions;
            # head 2w on partitions 0-63 of col w, head 2w+1 on 64-127) ----
            kt_sb = kvpool.tile([P, JQ, S], f16)
            for jk in range(JQ):
                ps = ps_p.tile([P, S], f32, tag="proj")
                for k in range(KT_TILES):
                    nc.tensor.matmul(
                        ps[:], kw_sb[:, k, jk * P:(jk + 1) * P], ctx_sb[:, k, :],
                        start=(k == 0), stop=(k == KT_TILES - 1))
                nc.scalar.activation(kt_sb[:, jk, :], ps[:], IDENT,
                                     bias=kb_sb[:, jk:jk + 1])

            # ---- qt(0) ----
            def emit_qt_slice(qt, xq, jq):
                ps = ps_p.tile([P, NCHUNK], f32, tag="proj")
                for k in range(KT_TILES):
                    nc.tensor.matmul(
                        ps[:], qw_sb[:, k, jq * P:(jq + 1) * P], xq[:, k, :],
                        start=(k == 0), stop=(k == KT_TILES - 1))
                if jq % 2 == 0:
                    nc.vector.tensor_scalar_add(qt[:, jq, :], ps[:],
                                                qb_sb[:, jq:jq + 1])
                else:
                    nc.scalar.activation(qt[:, jq, :], ps[:], IDENT,
                                         bias=qb_sb[:, jq:jq + 1])

            qt_cur = qtpool.tile([P, JQ, NCHUNK], f16, tag="qt")
            for jq in range(JQ):
                emit_qt_slice(qt_cur, xq_cur, jq)

            xq_next = emit_xq(1)

            # vbm = m01 (x) vb built on-device (saves 1MB of critical-head DMA)
            nc.gpsimd.partition_broadcast(vbb_sb[:], vb_sb[:])
            for st in range(ST):
                nc.vector.tensor_scalar_mul(vbm_sb[:, st, :], vbb_sb[:],
                                            m01_sb[:, st:st + 1])

            # ---- V = ctx @ vw -> [128(s), ST, HG, 65]; masked rows zeroed
            # (V = ps*m01 + m01*vb) and col D = m01 so psum row 64 of the AV
            # matmul accumulates the masked softmax denominator ----
            v_sb = kvpool.tile([P, ST, HG, D + 1], f16)
            for st in range(ST):
                ps = ps_p.tile([P, GF], f32, tag="proj")
                for k in range(KT_TILES):
                    nc.tensor.matmul(
                        ps[:], ctx_sb[:, k, st * P:(st + 1) * P], vw_sb[:, k, :],
                        start=(k == 0), stop=(k == KT_TILES - 1))
                nc.vector.scalar_tensor_tensor(
                    v_sb[:, st, :, 0:D],
                    ps.rearrange("p (h d) -> p h d", h=HG),
                    m01_sb[:, st:st + 1],
                    vbm_sb[:, st, :].rearrange("p (h d) -> p h d", h=HG),
                    mybir.AluOpType.mult, mybir.AluOpType.add)
                nc.scalar.activation(
                    v_sb[:, st, :, D], vbm_sb[:, st, 0:HG], IDENT,
                    bias=m01_sb[:, st:st + 1], scale=0.0)

            pw_r = pw.rearrange("(ko ki) m -> ki ko m", ki=P)
            nc.sync.dma_start(pw_sb[:], pw_r)

            def outproj_half(cc, ns, ot_src, fh, ostage):
                ps = ps_p.tile([P, DIM // 2], f32, tag="proj")
                for j in range(JQ):
                    nc.tensor.matmul(
                        ps[:],
                        ot_src[:, j, ns * P:(ns + 1) * P],
                        pw_sb[:, j, fh * 512:(fh + 1) * 512],
                        start=(j == 0), stop=(j == JQ - 1))
                if fh == 0:
                    nc.vector.tensor_copy(ostage[:, 0:512], ps[:])
                else:
                    nc.scalar.copy(ostage[:, 512:1024], ps[:])

            def out_dma(cc, ns, ostage):
                nc.sync.dma_start(
                    o[cc * NCHUNK + ns * P: cc * NCHUNK + (ns + 1) * P, :],
                    ostage[:])

            # ---- main chunk loop; out-proj (lagging by one wave-slice) and
            # q-proj of chunk c+1 are emitted INSIDE the attention waves so
            # the scheduler has independent full-mode matmuls to fill
            # scores->exp->AV stalls ----
            ot_hist = {}
            for c in range(NCH):
                ot_cur = otpool.tile([P, JQ, NCHUNK], f16, tag="ot")
                qt_next = None
                if c + 1 < NCH:
                    qt_next = qtpool.tile([P, JQ, NCHUNK], f16, tag="qt")
                for w in range(4):
                    h0 = 2 * w
                    # scores: per-st psum tile holds [A(st) | B(st)] so the
                    # head-pair MM duo is adjacent (row-tile-concurrent);
                    # two alternating slots (s0/s1) keep burst k+1's matmuls
                    # off burst k's exp critical path
                    e_p = epool.tile([P, ST, 2, NCHUNK], f16, tag="e")
                    for st in range(ST):
                        sT = ps_sc.tile([P, 2, NCHUNK], f32,
                                        tag=f"s{st % 2}")
                        nc.tensor.matmul(
                            sT[:, 0, :],
                            kt_sb[0:64, w, st * P:(st + 1) * P],
                            qt_cur[0:64, w, :], start=True, stop=True)
                        nc.tensor.matmul(
                            sT[:, 1, :],
                            kt_sb[64:128, w, st * P:(st + 1) * P],
                            qt_cur[64:128, w, :], start=True, stop=True)
                        nc.scalar.activation(
                            e_p[:, st, :, :], sT[:], EXP, scale=SCALE)

                    if c + 1 < NCH and w == 0:
                        xq_next2 = emit_xq(c + 2) if c + 2 < NCH else None

                    # out-proj slice shifted one wave late so the final
                    # chunk's normalize tail still has ready PE work
                    osl = (c - 1, w - 1) if w >= 1 else (c - 2, 3)

                    # filler while exp(A) drains
                    ostage = None
                    if osl[0] >= 0:
                        ostage = ostpool.tile([P, DIM], f16, tag="ostage")
                        outproj_half(osl[0], osl[1], ot_hist[osl[0]], 0, ostage)

                    # AV + masked denominator (psum row 64); st-granular deps
                    # let these chase the two exp instructions
                    opsA = ps_av.tile([D + 1, NCHUNK], f32, tag="av")
                    for st in range(ST):
                        nc.tensor.matmul(opsA[:], v_sb[:, st, h0, :],
                                         e_p[:, st, 0, :],
                                         start=(st == 0), stop=(st == ST - 1))
                    if osl[0] >= 0:
                        outproj_half(osl[0], osl[1], ot_hist[osl[0]], 1, ostage)
                        out_dma(osl[0], osl[1], ostage)
                    opsB = ps_av.tile([D + 1, NCHUNK], f32, tag="av")
                    for st in range(ST):
                        nc.tensor.matmul(opsB[:], v_sb[:, st, h0 + 1, :],
                                         e_p[:, st, 1, :],
                                         start=(st == 0), stop=(st == ST - 1))

                    if c + 1 < NCH:
                        emit_qt_slice(qt_next, xq_next, w)

                    # normalize: den -> 1/den -> broadcast -> ot fp16
                    # (both bounces on DVE: an ACT-queue copy would delay
                    # the next wave's exps in the strict-FIFO ACT queue)
                    dA = nmpool.tile([1, NCHUNK], f32, tag="dA")
                    nc.vector.tensor_copy(dA[:], opsA[D:D + 1, :])
                    dB = nmpool.tile([1, NCHUNK], f32, tag="dB")
                    nc.vector.tensor_copy(dB[:], opsB[D:D + 1, :])
                    rA = nmpool.tile([1, NCHUNK], f32, tag="rA")
                    nc.vector.reciprocal_approx_fast(rA[:], dA[:])
                    rB = nmpool.tile([1, NCHUNK], f32, tag="rB")
                    nc.vector.reciprocal_approx_fast(rB[:], dB[:])
                    rbA = nmpool.tile([P, NCHUNK], f32, tag="rbA")
                    nc.gpsimd.partition_broadcast(rbA[:], rA[:])
                    rbB = nmpool.tile([P, NCHUNK], f32, tag="rbB")
                    nc.gpsimd.partition_broadcast(rbB[:], rB[:])
                    nc.vector.tensor_mul(ot_cur[0:D, w, :], opsA[0:D, :],
                                         rbA[0:D, :])
                    nc.vector.tensor_mul(ot_cur[D:P, w, :], opsB[0:D, :],
                                         rbB[D:P, :])

                ot_hist[c] = ot_cur
                qt_cur = qt_next
                if c + 1 < NCH:
                    xq_cur = xq_next
                    xq_next = xq_next2

            for cc, ns in [(NCH - 2, 3), (NCH - 1, 0), (NCH - 1, 1),
                           (NCH - 1, 2), (NCH - 1, 3)]:
                ostage = ostpool.tile([P, DIM], f16, tag="ostage")
                outproj_half(cc, ns, ot_hist[cc], 0, ostage)
                outproj_half(cc, ns, ot_hist[cc], 1, ostage)
                out_dma(cc, ns, ostage)

    nc.compile()
    return nc


def _get_nc():
    global _CACHED_NC
    if _CACHED_NC is None:
        _CACHED_NC = _build()
    return _CACHED_NC


def kernel(x, context, context_mask, q_w, q_b, kv_w, kv_b, proj_w, proj_b):
    global LAST_RESULTS
    from concourse.bass_utils import run_bass_kernel_spmd

    x = np.asarray(x, dtype=np.float32)
    context = np.asarray(context, dtype=np.float32)
    context_mask = np.asarray(context_mask)
    q_w = np.asarray(q_w, dtype=np.float32)
    q_b = np.asarray(q_b, dtype=np.float32)
    kv_w = np.asarray(kv_w, dtype=np.float32)
    kv_b = np.asarray(kv_b, dtype=np.float32)
    proj_w = np.asarray(proj_w, dtype=np.float32)
    proj_b = np.asarray(proj_b, dtype=np.float32)

    c = np.ascontiguousarray

    in_maps = []
    for dev in range(8):
        b, g = dev // 2, dev % 2
        gs = g * GF
        m01_np = np.where(context_mask[b], np.float32(0.0), np.float32(1.0))
        h16 = np.float16
        in_maps.append({
            "xT": c(x[b].T.astype(h16)),
            "ctxT": c(context[b].T.astype(h16)),
            "qw": c(q_w[:, gs:gs + GF].astype(h16)),
            "kw": c(kv_w[:, gs:gs + GF].astype(h16)),
            "vw": c(kv_w[:, DIM + gs:DIM + gs + GF].astype(h16)),
            "pw": c(proj_w[gs:gs + GF, :].astype(h16)),
            "qb": c(q_b[gs:gs + GF].reshape(GF // P, P).T),
            "kb": c(kv_b[gs:gs + GF].reshape(GF // P, P).T),
            "vb": c(kv_b[DIM + gs:DIM + gs + GF].reshape(1, GF).astype(np.float32)),
            "m01": c(m01_np.reshape(S // P, P).T),
        })

    nc = _get_nc()
    try:
        res = run_bass_kernel_spmd(nc, in_maps, core_ids=list(range(8)))
    except Exception:
        # transient NRT_EXEC_UNIT_UNRECOVERABLE has been observed on a wedged
        # core; a straight retry recovers it
        res = run_bass_kernel_spmd(nc, in_maps, core_ids=list(range(8)))
    LAST_RESULTS = res

    out = np.empty((B, N, DIM), dtype=np.float32)
    for b in range(B):
        out[b] = (res.results[2 * b]["o"].astype(np.float32)
                  + res.results[2 * b + 1]["o"].astype(np.float32) + proj_b)
    return out
